# revision 32
# baseline (speedup 1.0000x reference)
"""Multi-head attention (B=2, S=2048, D=2048, H=16, Dh=128) on 8 TRN2 NeuronCores.

Tensor-parallel over heads: core c owns heads {2c, 2c+1}.

Per-core pipeline (bf16 data path, f32 PSUM/softmax):
  Phase A: QKV projection from replicated x^T, loaded as 1024-token pair
           tiles (2KB DMA lines — per-queue DMA rate scales with line
           length). The startup-critical wqkv+x stream is split 3 ways
           per chunk and striped in consumption order over the 3 DMA
           queues (sync/scalar/gpsimd); t=0 runs dc-major — 8 concurrent
           PSUM accumulations, one per bank — so the PE computes on each
           ~320KB chunk-pair as it lands. Q^T/K^T produced in [head_dim,
           token] layout (softmax scale folded into w_q on host); V
           natural [token, head_dim]. Batch 0 runs stand-alone; batch
           1's projection is interleaved into the first 32 attention
           super-slots.
  Phase B: one continuous 144-super-slot pipeline over BOTH local heads
           (global q-tile 0..15, head = qi//8): S^T[k,q] tile pairs via
           K^T-stationary matmuls into a 2-bank PSUM tile; one fused exp
           per pair on ScalarE straight out of PSUM (no max subtraction --
           logits are N(0,1)-scaled). PV^T accumulation trails exp by 2
           super-slots; a VectorE add-tree builds the softmax denominator
           and the flush runs one all-ones [128,128] matmul (row-sum
           broadcast to all partitions), a DVE reciprocal, and the 1/l
           comb normalization, streaming each [128,512] comb tile to the
           A2A buffer immediately. A2A(h0) triggers at the 8th flush,
           A2A(h1) at the 16th. All 16 phase-C even partials are emitted
           in the pipeline's drain slots. The loop is split at s=32 so
           the wqkv/x^T pools can close and phase C's SBUF opens in their
           space.
  Phase C: out-projection for the core's 512 tokens from 16 resident
           1024-wide w_out^T tiles (loaded once into an 8MB pool after
           phase A's SBUF closes — no reloads). Odd-head partials (second
           A2A) are added on VectorE and stored bf16 (host upcasts).

Host: shards/transposes weights (bf16), replicates x^T, concatenates per-core
token slices into the full (2, 2048, 2048) float32 output.
"""

import sys

import ml_dtypes
import numpy as np

for _p in ("/opt/trn_rl_repo", "/root/.axon_site/_ro/trn_rl_repo"):
    if _p not in sys.path:
        sys.path.insert(0, _p)

from concourse import bacc, bass, mybir, tile
from concourse.bass_utils import run_bass_kernel_spmd

B = 2
S = 2048
D = 2048
H = 16
DH = 128
NC = 8
HL = 2  # heads per core
T = B * S  # 4096 tokens
TPC = T // NC  # 512 tokens per core

F32 = mybir.dt.float32
F32R = mybir.dt.float32r
BF16 = mybir.dt.bfloat16
EXP = mybir.ActivationFunctionType.Exp

_graph_cache = {}


def build_graph(mm_dt=BF16):
    nc = bacc.Bacc(
        "TRN2",
        target_bir_lowering=False,
        debug=False,
        enable_asserts=False,
        num_devices=NC,
    )
    xT = nc.dram_tensor("xT", [D, T], BF16, kind="ExternalInput")
    ones_sq_in = nc.dram_tensor("ones_sq_in", [128, 128], BF16, kind="ExternalInput")
    wqkvT = nc.dram_tensor("wqkvT", [D, 3 * HL * DH], BF16, kind="ExternalInput")
    woutT = nc.dram_tensor("woutT", [D, D], BF16, kind="ExternalInput")
    out_ext = nc.dram_tensor("out", [TPC, D], BF16, kind="ExternalOutput")

    DC = D // 128  # 16 contraction chunks of 128
    n_k = S // 128  # 16 k tiles per (b, head)
    n_p = n_k // 2  # 8 kt-pairs per q-tile
    n_qt = B * (S // 512)  # 8 q-tiles per head

    with tile.TileContext(nc) as tc:
        with (
            tc.tile_pool(name="constp", bufs=1) as constp,
            tc.tile_pool(name="dramp", bufs=1, space="DRAM") as dramp,
        ):
            ones_sq = constp.tile([128, 128], BF16)

            a2a_send = [
                dramp.tile([NC, 128, TPC], BF16, name=f"a2a_send{h}") for h in range(HL)
            ]
            a2a_recv = [
                dramp.tile([NC, 128, TPC], BF16, name=f"a2a_recv{h}") for h in range(HL)
            ]

            with (
                tc.tile_pool(name="qkvp", bufs=1) as qkvp,
                tc.tile_pool(name="psA", bufs=2, space="PSUM") as psA,
            ):
                # persistent activations, split by batch so attention on b=0
                # only depends on the first half of the projection
                QT = [qkvp.tile([128, HL, S], mm_dt, name=f"QT{b}") for b in range(B)]
                KT = [qkvp.tile([128, HL, S], mm_dt, name=f"KT{b}") for b in range(B)]
                V = [
                    qkvp.tile([128, S // 128, HL * DH], mm_dt, name=f"V{b}")
                    for b in range(B)
                ]

                # w_out^T in 1024-wide tiles (2KB DMA lines), all 16 kept
                # resident in a dedicated pool opened once phase A's x/wqkv
                # SBUF closes — no mid-phase-C reloads. DMAs striped over 3
                # queues; emission order matches even-then-odd consumption.
                weighth2 = {}

                def emit_wout():
                    # NOT on the scalar queue: these triggers carry WAR
                    # waits on the freed phase-A SBUF, and a waiting trigger
                    # blocks every later exp in the scalar engine's FIFO.
                    wengs = (nc.gpsimd, nc.sync)
                    i = 0
                    for half in range(2):
                        for g2 in range(2):
                            for e in range(4 * half, 4 * half + 4):
                                wtile = pools["woutp"].tile(
                                    [128, 2, 1024], BF16, tag="wout", name="wout",
                                    bufs=16,
                                )
                                wengs[i % 2].dma_start(
                                    out=wtile[:],
                                    in_=woutT.ap()[
                                        e * 256 : (e + 1) * 256,
                                        g2 * 1024 : (g2 + 1) * 1024,
                                    ].rearrange("(dc p) f -> p dc f", p=128),
                                )
                                weighth2[(g2, e)] = wtile
                                i += 1

                def wout_rhs(g, e, dc2):
                    # [128, 512] rhs slice for dout group g, cin block
                    # (e, dc2): tile (g//2, e), chunk dc2, column half g%2
                    o = (g % 2) * 512
                    return weighth2[(g // 2, e)][:, dc2, o : o + 512]

                with (
                    tc.tile_pool(name="pB", bufs=2) as pB,
                    tc.tile_pool(name="psB", bufs=2, space="PSUM") as psB,
                ):
                    pending = []
                    nflush = [0]
                    ENGQ = (nc.sync, nc.gpsimd, nc.scalar)

                    def emit_a2a(hl):
                        # redistribute head->token sharding; shards were
                        # streamed per q-tile by the flushes.
                        nc.gpsimd.collective_compute(
                            "AllToAll",
                            mybir.AluOpType.bypass,
                            replica_groups=[list(range(NC))],
                            ins=[a2a_send[hl][:]],
                            outs=[a2a_recv[hl][:]],
                        )

                    def flush_pending():
                        # one all-ones [128,128] stationary matmul produces
                        # the denominator row-sum already broadcast to all
                        # 128 partitions; reciprocal runs on DVE.
                        ps_o_p, osum_p, hl_p, qi_p = pending.pop(0)
                        ps_lb = psA.tile([128, 512], F32, tag="psA", name="ps_lb")
                        nc.tensor.matmul(
                            ps_lb[:], ones_sq[:], osum_p[:], start=True, stop=True
                        )
                        rl = pB.tile([128, 512], F32, tag="rlf", bufs=2, name="rl")
                        nc.vector.reciprocal_approx_fast(out=rl[:], in_=ps_lb[:])
                        comb = pB.tile(
                            [128, 512], BF16, tag="comb", bufs=3, name="comb"
                        )
                        nc.vector.tensor_mul(comb[:], ps_o_p[:], rl[:])
                        # stream this q-tile's A2A shard immediately
                        nc.sync.dma_start(
                            out=a2a_send[hl_p][qi_p], in_=comb[:]
                        )
                        nflush[0] += 1
                        if nflush[0] == n_qt:
                            emit_a2a(0)
                        elif nflush[0] == 2 * n_qt:
                            emit_a2a(1)

                    # one continuous pipeline over both heads: global
                    # q-tile qi in 0..15, head = qi//8 — no per-head
                    # drain/refill seam, and A2A(h0) triggers as soon as
                    # its 8th q-tile flushes (mid-head-1).
                    NQ = 2 * n_qt  # 16 q-tiles across both heads
                    NS = NQ * n_p  # 128 super-slots
                    comb_in = [None] * DC
                    st = [None] * NQ
                    partials = {}
                    pools = {}  # late-bound: pools["pCX"] set after split

                    def emit_even_group(k):
                        # phase-C even-head partial for output tile
                        # (g, ts) = (k//4, k%4); evicted bf16 to SBUF.
                        g, ts_ = k // 4, k % 4
                        psE = psA.tile([128, 512], F32, tag="psA", name="psE")
                        for cc in range(8):
                            nc.tensor.matmul(
                                psE[:],
                                comb_in[cc][:, ts_ * 128 : (ts_ + 1) * 128],
                                wout_rhs(g, cc // 2, cc % 2),
                                start=(cc == 0),
                                stop=(cc == 7),
                            )
                        pev = pools["pCX"].tile(
                            [128, 512], BF16, tag="pev", bufs=16, name="pev"
                        )
                        nc.scalar.copy(pev[:], psE[:])
                        partials[(g, ts_)] = pev

                    def slot_body(s):
                        # ---- prefetch even-half comb_in once head-1 is
                        # underway: the sync queue reaches this point after
                        # the A2A(h0) data has landed, so phase C's even
                        # partials can start in the pipeline's drain.
                        if s == 104:
                            # gpsimd ONLY: these triggers wait on A2A(h0),
                            # and a late collective (cross-core skew) must
                            # not block the scalar queue's exps or the sync
                            # queue's flush sends. gpsimd has nothing urgent
                            # until A2A(h1).
                            for cc in range(DC // 2):
                                ctile = pools["pCX"].tile(
                                    [128, TPC], BF16, tag="comb_in",
                                    name="comb_in", bufs=DC,
                                )
                                nc.gpsimd.dma_start(
                                    out=ctile[:], in_=a2a_recv[0][cc]
                                )
                                comb_in[cc] = ctile
                        # ---- S^T pair + fused exp
                        if s < NS:
                            qi, pr = s // n_p, s % n_p
                            hl, qi_h = qi // n_qt, qi % n_qt
                            b = qi_h // 4
                            lsl = slice((qi_h % 4) * 512, (qi_h % 4 + 1) * 512)
                            if pr == 0:
                                st[qi] = {
                                    "ps_o": psB.tile(
                                        [128, 512], F32, tag="ps_o",
                                        name="ps_o",
                                    ),
                                    "pt2": [None] * n_p,
                                    "ptsums": [None] * n_p,
                                    "qsums": [None] * (n_p // 2),
                                    "hsums": [None] * 2,
                                }
                            ps_s = psB.tile(
                                [128, 1024], F32, tag="ps_s2", bufs=2,
                                name="ps_s",
                            )
                            for h in range(2):
                                kt = 2 * pr + h
                                nc.tensor.matmul(
                                    ps_s[:, h * 512 : (h + 1) * 512],
                                    KT[b][:, hl, kt * 128 : (kt + 1) * 128],
                                    QT[b][:, hl, lsl],
                                    start=True,
                                    stop=True,
                                )
                            pt2 = pB.tile(
                                [128, 1024], mm_dt, tag="pt", bufs=5,
                                name="pt2",
                            )
                            nc.scalar.activation(pt2[:], ps_s[:], EXP)
                            st[qi]["pt2"][pr] = pt2
                        # ---- PV^T pair (trails by 2)
                        if 2 <= s < NS + 2:
                            s2 = s - 2
                            qi, pr = s2 // n_p, s2 % n_p
                            hl, qi_h = qi // n_qt, qi % n_qt
                            b = qi_h // 4
                            for h in range(2):
                                kt = 2 * pr + h
                                nc.tensor.matmul(
                                    st[qi]["ps_o"][:],
                                    V[b][:, kt, hl * DH : (hl + 1) * DH],
                                    st[qi]["pt2"][pr][
                                        :, h * 512 : (h + 1) * 512
                                    ],
                                    start=(kt == 0),
                                    stop=(kt == n_k - 1),
                                )
                        # ---- DVE reduction tree for the denominator
                        if 2 <= s < NS + 2:
                            gp = s - 2
                            qi, j = gp // n_p, gp % n_p
                            psm = pB.tile(
                                [128, 512], mm_dt, tag="ptsum", bufs=5,
                                name="psm",
                            )
                            nc.vector.tensor_add(
                                psm[:],
                                st[qi]["pt2"][j][:, 0:512],
                                st[qi]["pt2"][j][:, 512:1024],
                            )
                            st[qi]["ptsums"][j] = psm
                        if 4 <= s < NS + 4 and s % 2 == 0:
                            gq = (s - 4) // 2
                            qi, j2 = gq // (n_p // 2), gq % (n_p // 2)
                            qsm = pB.tile(
                                [128, 512], mm_dt, tag="qsum", bufs=4,
                                name="qsm",
                            )
                            nc.vector.tensor_add(
                                qsm[:],
                                st[qi]["ptsums"][2 * j2][:],
                                st[qi]["ptsums"][2 * j2 + 1][:],
                            )
                            st[qi]["qsums"][j2] = qsm
                        if s >= 11 and (s - 11) % n_p == 0 and (s - 11) // n_p < NQ:
                            qi = (s - 11) // n_p
                            hs = pB.tile(
                                [128, 512], mm_dt, tag="hsum", bufs=2,
                                name="hs0",
                            )
                            nc.vector.tensor_add(
                                hs[:], st[qi]["qsums"][0][:], st[qi]["qsums"][1][:]
                            )
                            st[qi]["hsums"][0] = hs
                        if s >= 12 and (s - 12) % n_p == 0 and (s - 12) // n_p < NQ:
                            qi = (s - 12) // n_p
                            hs = pB.tile(
                                [128, 512], mm_dt, tag="hsum", bufs=2,
                                name="hs1",
                            )
                            nc.vector.tensor_add(
                                hs[:], st[qi]["qsums"][2][:], st[qi]["qsums"][3][:]
                            )
                            st[qi]["hsums"][1] = hs
                        if s >= 13 and (s - 13) % n_p == 0 and (s - 13) // n_p < NQ:
                            qi = (s - 13) // n_p
                            osum = pB.tile(
                                [128, 512], mm_dt, tag="osum", bufs=2,
                                name="osum",
                            )
                            nc.vector.tensor_add(
                                osum[:],
                                st[qi]["hsums"][0][:],
                                st[qi]["hsums"][1][:],
                            )
                            st[qi]["osum"] = osum
                            pending.append(
                                (st[qi]["ps_o"], osum, qi // n_qt, qi % n_qt)
                            )
                        if s % n_p == 0 and s > 0 and pending:
                            flush_pending()
                        # ---- fill the drain window: these sit after the
                        # last flush matmul in PE order, so they delay no
                        # send, but run as soon as its operand is ready.
                        if NS + 9 <= s:
                            emit_even_group(s - (NS + 9))

                    with (
                        tc.tile_pool(name="scrA", bufs=1) as scrA,
                        tc.tile_pool(name="xtp", bufs=8) as xtp,
                    ):
                        wqkv_s = scrA.tile([128, DC, 3 * HL * DH], mm_dt)

                        def emit_xq_half(xq, tp, h2):
                            """128KB chunk DMAs for token-half h2 of pair tp,
                            round-robin over the queues."""
                            engs = ENGQ
                            for qh in range(4):
                                for dcq in range(4):
                                    eng = engs[(qh + dcq) % 3]
                                    eng.dma_start(
                                        out=xq[qh][
                                            :, dcq, h2 * 512 : (h2 + 1) * 512
                                        ],
                                        in_=xT.ap()[
                                            qh * 512 + dcq * 128 : qh * 512
                                            + (dcq + 1) * 128,
                                            tp * 1024 + h2 * 512 : tp * 1024
                                            + (h2 + 1) * 512,
                                        ].rearrange(
                                            "(dc p) f -> p dc f", p=128
                                        )[:, 0, :],
                                    )

                        def alloc_xq():
                            return [
                                xtp.tile([128, 4, 1024], mm_dt, tag="xq", name="xq")
                                for _ in range(4)
                            ]

                        def emit_xq_pair(tp):
                            """Load x^T token-pair tp (1024 tokens, 2KB DMA
                            lines) as 4 quarter tiles [128, 4, 1024]."""
                            engs = ENGQ
                            xq = alloc_xq()
                            for qh in range(4):
                                engs[qh % 3].dma_start(
                                    out=xq[qh][:],
                                    in_=xT.ap()[
                                        qh * 512 : (qh + 1) * 512,
                                        tp * 1024 : (tp + 1) * 1024,
                                    ].rearrange("(dc p) f -> p dc f", p=128),
                                )
                            return xq

                        def emit_a_group(t, gi, xq):
                            """One projection group: gi<4 -> Q/K column group
                            gi for 512 tokens; gi>=4 -> V rows for token
                            sub-tile gi-4. xq is the t//2 pair; h2 selects
                            the token half."""
                            h2 = t % 2
                            if gi < 4:
                                ps = psA.tile(
                                    [128, 512], F32, tag="psA", name="psqk"
                                )
                                for dc in range(DC):
                                    nc.tensor.matmul(
                                        ps[:],
                                        wqkv_s[:, dc, gi * 128 : (gi + 1) * 128],
                                        xq[dc // 4][
                                            :, dc % 4,
                                            h2 * 512 : (h2 + 1) * 512,
                                        ],
                                        start=(dc == 0),
                                        stop=(dc == DC - 1),
                                    )
                                dest = QT if gi < HL else KT
                                hl = gi % HL
                                nc.vector.tensor_copy(
                                    dest[t // 4][
                                        :, hl, (t % 4) * 512 : (t % 4 + 1) * 512
                                    ],
                                    ps[:],
                                )
                            else:
                                sub = gi - 4
                                psv = psA.tile(
                                    [128, HL * DH], F32, tag="psA", name="psv"
                                )
                                for dc in range(DC):
                                    nc.tensor.matmul(
                                        psv[:],
                                        xq[dc // 4][
                                            :, dc % 4,
                                            h2 * 512 + sub * 128 : h2 * 512
                                            + (sub + 1) * 128,
                                        ],
                                        wqkv_s[:, dc, 2 * HL * DH : 3 * HL * DH],
                                        start=(dc == 0),
                                        stop=(dc == DC - 1),
                                    )
                                nc.vector.tensor_copy(
                                    V[t // 4][:, (t % 4) * 4 + sub, :], psv[:]
                                )

                        # ---------------- Phase A: projection of b=0 --------
                        # Startup-critical stream: wqkv chunk dc and x^T
                        # piece dc are emitted adjacently in consumption
                        # order, striped over all 4 DMA queues, so t=0 can
                        # run dc-major (8 concurrent PSUM accumulations) on
                        # each ~320KB chunk-pair as it lands.
                        xq_pairs = {}
                        xq_pairs[0] = alloc_xq()
                        # each chunk is split 3 ways (wqkv partition halves +
                        # x piece) across all 3 queues, so every ~320KB
                        # chunk-pair lands 3x faster than single-queue and
                        # the dc-major consumer never starves.
                        for dc in range(DC):
                            r = dc % 3
                            for ph in range(2):
                                ENGQ[(r + ph) % 3].dma_start(
                                    out=wqkv_s[
                                        ph * 64 : (ph + 1) * 64, dc : dc + 1, :
                                    ],
                                    in_=wqkvT.ap()[
                                        dc * 128 + ph * 64 : dc * 128
                                        + (ph + 1) * 64,
                                        :,
                                    ].rearrange("(dc p) f -> p dc f", p=64),
                                )
                            ENGQ[(r + 2) % 3].dma_start(
                                out=xq_pairs[0][dc // 4][:, dc % 4, 0:512],
                                in_=xT.ap()[
                                    dc * 128 : (dc + 1) * 128, 0:512
                                ].rearrange("(dc p) f -> p dc f", p=128)[:, 0, :],
                            )
                        nc.scalar.dma_start(out=ones_sq[:], in_=ones_sq_in.ap())
                        emit_xq_half(xq_pairs[0], 0, 1)
                        xq_pairs[1] = emit_xq_pair(1)

                        # t=0 dc-major: phase B hasn't started, so its PSUM
                        # holds the 8 accumulators (Q/K pairs in the ps_s2
                        # [128,1024] tiles, V sub-tiles packed 2x[128,256]
                        # into the ps_o tiles).
                        qk_acc = [
                            psB.tile(
                                [128, 1024], F32, tag="ps_s2", bufs=2,
                                name="qk_acc",
                            )
                            for _ in range(2)
                        ]
                        # each V sub-tile needs its OWN bank: start=True
                        # clears has_written for the whole bank, so two
                        # accumulation groups must never share one.
                        v_acc = [
                            psB.tile([128, 256], F32, tag="ps_o", name="v_acc")
                            for _ in range(2)
                        ] + [
                            psA.tile([128, 256], F32, tag="psA", name="v_accA")
                            for _ in range(2)
                        ]
                        for dc in range(DC):
                            for gi in range(4):
                                nc.tensor.matmul(
                                    qk_acc[gi // 2][
                                        :, (gi % 2) * 512 : (gi % 2 + 1) * 512
                                    ],
                                    wqkv_s[:, dc, gi * 128 : (gi + 1) * 128],
                                    xq_pairs[0][dc // 4][:, dc % 4, 0:512],
                                    start=(dc == 0),
                                    stop=(dc == DC - 1),
                                )
                            for sub in range(4):
                                nc.tensor.matmul(
                                    v_acc[sub][:],
                                    xq_pairs[0][dc // 4][
                                        :, dc % 4, sub * 128 : (sub + 1) * 128
                                    ],
                                    wqkv_s[:, dc, 2 * HL * DH : 3 * HL * DH],
                                    start=(dc == 0),
                                    stop=(dc == DC - 1),
                                )
                        for gi in range(4):
                            dest = QT if gi < HL else KT
                            nc.vector.tensor_copy(
                                dest[0][:, gi % HL, 0:512],
                                qk_acc[gi // 2][
                                    :, (gi % 2) * 512 : (gi % 2 + 1) * 512
                                ],
                            )
                        for sub in range(4):
                            nc.vector.tensor_copy(
                                V[0][:, sub, :], v_acc[sub][:]
                            )

                        for t in range(1, 4):
                            if t == 2:
                                xq_pairs[2] = emit_xq_pair(2)
                            for gi in range(8):
                                emit_a_group(t, gi, xq_pairs[t // 2])
                        del xq_pairs[0], xq_pairs[1]

                        # -------- Phase B slots 0..31 (A(b=1) interleave) ---
                        xq_pairs[3] = emit_xq_pair(3)
                        for s in range(32):
                            t = 4 + s // 8
                            emit_a_group(t, s % 8, xq_pairs[2 + (s // 16)])
                            slot_body(s)

                    # wqkv/x^T pools are closed: phase C's SBUF (comb_in,
                    # partials, the full resident w_out^T) opens in the
                    # freed space for slots 32+.
                    with (
                        tc.tile_pool(name="pCX", bufs=1) as pCX,
                        tc.tile_pool(name="woutp", bufs=1) as woutp2,
                    ):
                        pools["pCX"] = pCX
                        pools["woutp"] = woutp2
                        emit_wout()
                        # drain slots emit all 16 even-half out-proj groups
                        for s in range(32, NS + 25):
                            slot_body(s)
                        while pending:
                            flush_pending()

                        # ---------------- Phase C: out projection ----------
                        # odd-half comb_in (gated on A2A h1). NOT on the
                        # scalar queue: these triggers wait for the
                        # collective, and the tile scheduler can place them
                        # ahead of the drain's pev evictions, stalling the
                        # PE ~10us. cc-order on gpsimd/sync matches the
                        # dc-major consumption below.
                        for cc in range(DC // 2, DC):
                            ctile = pCX.tile(
                                [128, TPC], BF16, tag="comb_in", name="comb_in",
                                bufs=DC,
                            )
                            (nc.gpsimd, nc.sync)[cc % 2].dma_start(
                                out=ctile[:], in_=a2a_recv[1][cc - DC // 2]
                            )
                            comb_in[cc] = ctile
                        # odd partials run dc-major in two 8-bank waves so
                        # the PE consumes each comb_in chunk as it lands
                        # instead of waiting for all 8 post-collective DMAs.
                        for wave in range(2):
                            s2w = [
                                psB.tile(
                                    [128, 1024], F32, tag="ps_s2", bufs=2,
                                    name="w_s2",
                                )
                                for _ in range(2)
                            ]
                            ow = [
                                psB.tile(
                                    [128, 512], F32, tag="ps_o", bufs=2,
                                    name="w_o",
                                )
                                for _ in range(2)
                            ]
                            aw = [
                                psA.tile(
                                    [128, 512], F32, tag="psA", name="w_a"
                                )
                                for _ in range(2)
                            ]
                            accs = [
                                s2w[0][:, 0:512],
                                s2w[0][:, 512:1024],
                                s2w[1][:, 0:512],
                                s2w[1][:, 512:1024],
                                ow[0][:],
                                ow[1][:],
                                aw[0][:],
                                aw[1][:],
                            ]
                            for cc in range(8, DC):
                                for j in range(8):
                                    g, ts = 2 * wave + j // 4, j % 4
                                    nc.tensor.matmul(
                                        accs[j],
                                        comb_in[cc][:, ts * 128 : (ts + 1) * 128],
                                        wout_rhs(g, cc // 2, cc % 2),
                                        start=(cc == 8),
                                        stop=(cc == DC - 1),
                                    )
                            for j in range(8):
                                g, ts = 2 * wave + j // 4, j % 4
                                ev = pCX.tile(
                                    [128, 512], BF16, tag="ev", bufs=4
                                )
                                nc.vector.tensor_add(
                                    ev[:], accs[j], partials[(g, ts)][:]
                                )
                                # stripe stores over sync+gpsimd (NOT
                                # scalar); serializing 8 on one queue adds
                                # ~2.5us after the last matmul.
                                (nc.sync, nc.gpsimd)[j % 2].dma_start(
                                    out=out_ext.ap()[
                                        ts * 128 : (ts + 1) * 128,
                                        g * 512 : (g + 1) * 512,
                                    ],
                                    in_=ev[:],
                                )
    nc.finalize()
    return nc


def prep_inputs(x, w_qkv, w_out):
    """Host-side sharding. Returns list of per-core input dicts."""
    x = np.asarray(x, dtype=np.float32)
    w_qkv = np.asarray(w_qkv, dtype=np.float32)
    w_out = np.asarray(w_out, dtype=np.float32)

    xT = np.ascontiguousarray(x.reshape(T, D).T).astype(ml_dtypes.bfloat16)

    # w_out^T with rows permuted to (even heads | odd heads)
    woutT = w_out.T  # [cin, dout], cin = h*DH + d
    perm = [2 * i for i in range(8)] + [2 * i + 1 for i in range(8)]
    woutT_bf = np.ascontiguousarray(
        np.concatenate([woutT[h * DH : (h + 1) * DH] for h in perm], axis=0)
    ).astype(ml_dtypes.bfloat16)

    scale = np.float32(1.0 / np.sqrt(DH))
    ones_sq = np.ones((128, 128), dtype=ml_dtypes.bfloat16)
    in_maps = []
    for c in range(NC):
        h0 = HL * c
        wq = w_qkv[h0 * DH : (h0 + HL) * DH] * scale  # [256, D]
        wk = w_qkv[H * DH + h0 * DH : H * DH + (h0 + HL) * DH]
        wv = w_qkv[2 * H * DH + h0 * DH : 2 * H * DH + (h0 + HL) * DH]
        wqkvT = np.ascontiguousarray(np.concatenate([wq, wk, wv], axis=0).T).astype(
            ml_dtypes.bfloat16
        )
        in_maps.append(
            {
                "xT": xT,
                "wqkvT": wqkvT,
                "woutT": woutT_bf,
                "ones_sq_in": ones_sq,
            }
        )
    return in_maps


def run(x, w_qkv, w_out, mm_dt=BF16, trace=False, tmpdir=None):
    key = str(mm_dt)
    if key not in _graph_cache:
        _graph_cache[key] = build_graph(mm_dt)
    nc = _graph_cache[key]
    in_maps = prep_inputs(x, w_qkv, w_out)
    res = run_bass_kernel_spmd(
        nc, in_maps, core_ids=list(range(NC)), trace=trace, tmpdir=tmpdir
    )
    out = np.concatenate([res.results[c]["out"] for c in range(NC)], axis=0)
    return out.reshape(B, S, D).astype(np.float32), res


def kernel(x, w_qkv, w_out):
    out, _ = run(x, w_qkv, w_out)
    return out

